# revision 10
# baseline (speedup 1.0000x reference)
"""CTLSTM (continuous-time LSTM, state re-init variant) Trainium2 kernel, v2.

Key insight: the reference re-initializes h/c/c_bar to zero every timestep, so
the 7H gate pre-activations depend ONLY on the event type (1001 distinct
embedding rows), not on the token. Outputs c, c_bar, go, gd are therefore pure
per-type values; only h_d = go*tanh(c_bar + (c-c_bar)*exp(-gd*dur)) mixes in
the per-token duration.

Device (per core, tensor-parallel over H: core k owns H columns
[128k, 128k+128)):
  Phase 1: G = embT @ W slices (bf16 matmuls, bias via a K=1 ones-row
    matmul), then build a bf16 per-type table row [DIF | CB | GO | GD]
    (DIF = C-CB) living in SBUF with type t at partition t%128, rank t//128
    (the SBUF-source dma_gather stripe layout), and DMA one 512KB copy to
    DRAM for host-side assembly of the 4 pure-type outputs.
  Phase 2: 8 waves x 2048 tokens: SBUF-source dma_gather (transpose mode)
    -> g[128 H, 4 quant, 2048 tok]; duration broadcast across partitions via
    a K=1 PE matmul into PSUM; then DVE/ACT pointwise for h_d only, written
    as bf16 [128, 16384].

Host: re-assembles h_d (transpose + f32 cast) and gathers outputs 1-4 from
the per-type tables by event id (replication of device-computed values).
"""

import os

import numpy as np

HIDDEN = 1024
TYPES = 1001
TPAD = 1024          # padded type count (8 m-tiles of 128)
B = 32
T = 512
NTOK = B * T         # 16384
NCORES = 8
NGATES = 5           # i, z, o, ibar, d  (f, fbar unused by the reference)
GATE_ROWS = (0, 2, 3, 4, 6)  # row-group index of each used gate in W_rec/b_rec
NCOLS = NGATES * 128  # 640 gate columns per core
WAVE = 2048          # tokens per phase-2 wave
NWAVES = NTOK // WAVE
KT = HIDDEN // 128   # 8 contraction k-tiles

# Set KERNEL_TRACE=1 to capture an NTFF profile; the BassKernelResults of the
# last run is stashed in LAST_RESULTS.
LAST_RESULTS = None
_CACHED_NC = None


def _build_nc():
    import concourse.mybir as mybir
    from concourse import bacc
    from concourse.tile import TileContext

    dt = mybir.dt
    AF = mybir.ActivationFunctionType
    f32 = dt.float32
    f32r = dt.float32r
    bf16 = dt.bfloat16

    nc = bacc.Bacc("TRN2", target_bir_lowering=False, debug=False)

    et_d = nc.dram_tensor("et", [128, KT, TPAD], bf16, kind="ExternalInput")
    wt_d = nc.dram_tensor("wt", [128, KT, NCOLS], bf16, kind="ExternalInput")
    bias_d = nc.dram_tensor("bias", [1, NCOLS], bf16, kind="ExternalInput")
    idx_d = nc.dram_tensor("idx", [128, NTOK // 16], dt.int16, kind="ExternalInput")
    dur_d = nc.dram_tensor("durneg", [1, NTOK], f32r, kind="ExternalInput")
    hd_d = nc.dram_tensor("hd", [128, NTOK], bf16, kind="ExternalOutput")
    tbl_d = nc.dram_tensor("tbl", [128, TPAD // 128 * 512], bf16, kind="ExternalOutput")

    with TileContext(nc) as tc:
        with tc.tile_pool(name="const", bufs=1) as cpool:
            et_sb = cpool.tile([128, KT, TPAD], bf16, tag="et")
            nc.sync.dma_start(out=et_sb[:], in_=et_d[:])
            wt_sb = cpool.tile([128, KT, NCOLS], bf16, tag="wt")
            nc.sync.dma_start(out=wt_sb[:], in_=wt_d[:])
            bias_sb = cpool.tile([1, NCOLS], bf16, tag="bias")
            nc.sync.dma_start(out=bias_sb[:], in_=bias_d[:])
            idx_sb = cpool.tile([128, NTOK // 16], dt.int16, tag="idx")
            nc.sync.dma_start(out=idx_sb[:], in_=idx_d[:])
            dur_sb = cpool.tile([1, NTOK], f32r, tag="dur")
            nc.sync.dma_start(out=dur_sb[:], in_=dur_d[:])

            ones_bf = cpool.tile([1, 128], bf16, tag="onesb")
            nc.vector.memset(ones_bf[:], 1.0)
            ones_f = cpool.tile([1, 128], f32, tag="onesf")
            nc.vector.memset(ones_f[:], 1.0)

            # per-type gather table [DIF | CB | GO | GD], type t at
            # partition t%128, byte offset (t//128)*1024
            table = cpool.tile([128, TPAD // 128, 512], bf16, tag="table")
            gd_all = cpool.tile([128, TPAD // 128, 128], f32, tag="gdall")
            gd_exp = cpool.tile([128, TPAD // 128, 128], f32, tag="gdexp")

            # ---- phase 1: gate tables ------------------------------------
            with (
                tc.tile_pool(name="p1psum", bufs=2, space="PSUM") as ppool,
                tc.tile_pool(name="p1sb", bufs=2) as epool,
            ):
                for m in range(TPAD // 128):
                    psA = ppool.tile([128, 384], f32, tag="psA")  # gi|gz|go
                    psB = ppool.tile([128, 256], f32, tag="psB")  # gib|gd
                    for kt in range(KT):
                        lhs = et_sb[:, kt, m * 128:(m + 1) * 128]
                        first = kt == 0
                        nc.tensor.matmul(psA[:, :], lhs, wt_sb[:, kt, 0:384],
                                         start=first, stop=False)
                        nc.tensor.matmul(psB[:, :], lhs, wt_sb[:, kt, 384:640],
                                         start=first, stop=False)
                    # bias via K=1 ones-row matmul closing the group
                    nc.tensor.matmul(psA[:, :], ones_bf[0:1, :],
                                     bias_sb[0:1, 0:384], start=False, stop=True)
                    nc.tensor.matmul(psB[:, :], ones_bf[0:1, :],
                                     bias_sb[0:1, 384:640], start=False, stop=True)
                    gi = epool.tile([128, 128], f32, tag="gi")
                    tz = epool.tile([128, 128], f32, tag="tz")
                    gib = epool.tile([128, 128], f32, tag="gib")
                    dif = epool.tile([128, 128], f32, tag="dif")
                    nc.scalar.activation(out=gi[:], in_=psA[:, 0:128], func=AF.Sigmoid)
                    nc.scalar.activation(out=tz[:], in_=psA[:, 128:256], func=AF.Tanh)
                    nc.scalar.activation(out=table[:, m, 256:384],
                                         in_=psA[:, 256:384], func=AF.Sigmoid)
                    nc.scalar.activation(out=gib[:], in_=psB[:, 0:128], func=AF.Sigmoid)
                    nc.vector.tensor_copy(out=gd_all[:, m, :], in_=psB[:, 128:256])
                    nc.vector.tensor_sub(dif[:], gi[:], gib[:])
                    nc.vector.tensor_mul(table[:, m, 0:128], dif[:], tz[:])
                    nc.vector.tensor_mul(table[:, m, 128:256], gib[:], tz[:])
                # softplus(gd) = Ln(1 + Exp(gd))
                nc.scalar.activation(out=gd_exp[:], in_=gd_all[:], func=AF.Exp)
                nc.scalar.activation(out=table[:, :, 384:512], in_=gd_exp[:],
                                     func=AF.Ln, bias=1.0)
            nc.sync.dma_start(out=tbl_d[:], in_=table[:].rearrange("p m e -> p (m e)"))

            # ---- phase 2: gather + pointwise -----------------------------
            with (
                tc.tile_pool(name="p2psum", bufs=2, space="PSUM") as dpool,
                tc.tile_pool(name="wave", bufs=3) as wpool,
                tc.tile_pool(name="scratch", bufs=2) as spool,
            ):
                for w in range(NWAVES):
                    g = wpool.tile([128, 4, WAVE], bf16, tag="g")
                    nc.gpsimd.dma_gather(
                        g[:],
                        table[:],
                        idx_sb[:, w * (WAVE // 16):(w + 1) * (WAVE // 16)],
                        WAVE,
                        WAVE,
                        512,
                        transpose=True,
                        single_packet=False,
                        sbuf_tokens_per_rank=128,
                        sbuf_free_dim_per_rank=1024,
                    )
                    durB = dpool.tile([128, WAVE], f32, tag="durB")
                    for c in range(WAVE // 512):
                        c0 = w * WAVE + c * 512
                        nc.tensor.matmul(
                            durB[:, c * 512:(c + 1) * 512],
                            ones_f[0:1, :].bitcast(f32r),
                            dur_sb[0:1, c0:c0 + 512],
                            start=True, stop=True)
                    tmp = spool.tile([128, WAVE], f32, tag="tmp")
                    te = spool.tile([128, WAVE], f32, tag="te")
                    nc.vector.tensor_mul(tmp[:], g[:, 3, :], durB[:])
                    nc.scalar.activation(out=te[:], in_=tmp[:], func=AF.Exp)
                    nc.vector.tensor_mul(tmp[:], g[:, 0, :], te[:])
                    nc.vector.tensor_add(tmp[:], tmp[:], g[:, 1, :])
                    nc.scalar.activation(out=te[:], in_=tmp[:], func=AF.Tanh)
                    hdw = wpool.tile([128, WAVE], bf16, tag="hdw")
                    nc.vector.tensor_mul(hdw[:], g[:, 2, :], te[:])
                    nc.sync.dma_start(out=hd_d[:, w * WAVE:(w + 1) * WAVE], in_=hdw[:])

    nc.compile()
    return nc


def _ensure_ntff_hook():
    """The agent image's antenv lacks axon_hooks; shim it and register the
    ctypes NTFF profiling hook so trace=True works under axon."""
    import sys
    import types

    try:
        from antenv.axon_hooks import get_axon_ntff_profile_hook  # noqa: F401
        return
    except ImportError:
        pass
    try:
        import antenv
    except ImportError:
        return
    mod = types.ModuleType("antenv.axon_hooks")
    state = {"hook": None}
    mod.set_axon_ntff_profile_hook = lambda h: state.__setitem__("hook", h)
    mod.get_axon_ntff_profile_hook = lambda: state["hook"]
    sys.modules["antenv.axon_hooks"] = mod
    antenv.axon_hooks = mod
    try:
        from trn_agent_boot.trn_boot import _ntff_profile_via_ctypes

        hook = _ntff_profile_via_ctypes("/opt/axon/libaxon_pjrt.so")
        if hook is not None:
            mod.set_axon_ntff_profile_hook(hook)
    except Exception:
        pass


def kernel(event_seqs, duration_seqs, emb_table, W_rec, b_rec):
    global LAST_RESULTS, _CACHED_NC
    import ml_dtypes
    from concourse.bass_utils import run_bass_kernel_spmd

    bf16 = ml_dtypes.bfloat16
    ev = np.asarray(event_seqs)
    dur = np.asarray(duration_seqs, dtype=np.float32)
    emb = np.asarray(emb_table, dtype=np.float32)
    W = np.asarray(W_rec, dtype=np.float32)
    b = np.asarray(b_rec, dtype=np.float32)

    # ---- host-side input marshaling (sharding) -----------------------------
    embT = np.zeros((HIDDEN, TPAD), np.float32)
    embT[:, :TYPES] = emb.T
    et = np.ascontiguousarray(
        embT.reshape(KT, 128, TPAD).transpose(1, 0, 2)).astype(bf16)

    ev_tok = ev.T.reshape(-1).astype(np.int16)          # token t*32+b -> type
    # idx i at [i%16, i//16], replicated across the 8 GPSIMD core stripes
    idx = np.tile(ev_tok.reshape(-1, 16).T, (8, 1)).astype(np.int16)

    durneg = np.ascontiguousarray((-dur.T.reshape(-1)).reshape(1, NTOK))

    in_maps = []
    for k in range(NCORES):
        h0 = 128 * k
        wt = np.zeros((HIDDEN, NCOLS), np.float32)
        bias = np.zeros((1, NCOLS), np.float32)
        for g5, g7 in enumerate(GATE_ROWS):
            rows = slice(g7 * HIDDEN + h0, g7 * HIDDEN + h0 + 128)
            wt[:, g5 * 128:(g5 + 1) * 128] = W[rows, :HIDDEN].T
            bias[0, g5 * 128:(g5 + 1) * 128] = b[rows]
        wt = np.ascontiguousarray(
            wt.reshape(KT, 128, NCOLS).transpose(1, 0, 2)).astype(bf16)
        in_maps.append({"et": et, "wt": wt, "bias": bias.astype(bf16),
                        "idx": idx, "durneg": durneg})

    if _CACHED_NC is None:
        _CACHED_NC = _build_nc()
    nc = _CACHED_NC

    trace = os.environ.get("KERNEL_TRACE", "") not in ("", "0")
    if trace:
        _ensure_ntff_hook()
    res = run_bass_kernel_spmd(nc, in_maps, list(range(NCORES)), trace=trace)
    LAST_RESULTS = res

    # ---- host-side output assembly ----------------------------------------
    full = np.empty((5, NTOK, HIDDEN), np.float32)
    ev_flat = ev.T.reshape(-1).astype(np.int64)
    # per-quantity full-width type tables [TPAD, HIDDEN]
    qtbl = np.empty((4, TPAD, HIDDEN), np.float32)  # C, CB, GO, GD
    for k in range(NCORES):
        sl = slice(128 * k, 128 * (k + 1))
        o = res.results[k]["hd"]  # [128, NTOK] bf16
        full[0][:, sl] = o.T
        tbl = res.results[k]["tbl"].reshape(128, TPAD // 128, 512)
        tbl = tbl.transpose(1, 0, 2).reshape(TPAD, 512).astype(np.float32)
        qtbl[0, :, sl] = tbl[:, 0:128] + tbl[:, 128:256]  # C = DIF + CB
        qtbl[1, :, sl] = tbl[:, 128:256]
        qtbl[2, :, sl] = tbl[:, 256:384]
        qtbl[3, :, sl] = tbl[:, 384:512]
    for s in range(4):
        full[s + 1] = qtbl[s][ev_flat]
    return full.reshape(5, T, B, HIDDEN)
